# revision 24
# baseline (speedup 1.0000x reference)
"""Trainium2 Bass kernel for a pairwise-distance cluster margin loss.

Math (matches the jax reference):
    far_i  = max_{j: t_j=t_i} dist_ij
    near_i = second smallest dist_ij over class(i)  (smallest is self)
    loss   = mean(relu(far - near))

Key insight: the loss only involves SAME-CLASS distances.  With rows
sorted by class, each 128-row tile's class-mates lie within a narrow
band of the sorted order (max class size ~82), so each tile only needs
W ~ 264 columns instead of 4096 -> ~14x less GEMM work than the full
distance matrix.

The class order is annealed on the host so every 128-row window's
class band fits in [win-64, win+W-128) -- W ~ 208 instead of 4096.

Per core (512 sorted rows): the column "universe" is the sorted slice
order[512c-SPL : 512c-SPL+NCOL] (padded with zeros at the array ends).
Row-tile mt multiplies against universe cols [128mt, 128mt+W).  A single
fp8 tensor xt8 = fp8(sqrt2*x[universe])^T serves as BOTH matmul operands
(lhsT slice = own rows, rhs slice = window), so the PE computes
    psA = 2 x_i.x_j - sq_j - C*mask      (fp8 DR chain + one bf16 aug)
and the stats flip max<->min versus the usual formulation:
    rowmin(psA)                   -> far2  = sq_i - C - fstat
    rowmax(psA + 2C*mask + Ddiag) -> near2 = sq_i + C - gstat
The chains are issued chunk-major so the PE consumes chunk pairs in DMA
arrival order (inputs stream just-in-time on three engine queues); after
each tile's f-reduce, one eye-matmul accumulates the host-precomputed
2C*mask + DIAG*diag block into the same PSUM bank (WAR dependency), so
the near stat is a plain rowmax and the DVE does only two reduces per
tile.  Host applies sqrt / relu / mean on the 4096 reduced stats.
"""

import numpy as np
import ml_dtypes

BF = ml_dtypes.bfloat16
F8 = ml_dtypes.float8_e4m3

N = 4096  # rows (points)
D = 2048  # feature dim
P = 128  # partitions
NCORES = 8
MB = N // NCORES  # 512 rows per core
KX = D // P  # 16 x-chunks of 128
MT = MB // P  # 4 row tiles of 128 per core
NCLS = 64

C = float(2.0**17)  # mask offset; > max |2xixj - sqj| (~15k)
DIAG = -float(2.0**31)  # diagonal push-out

_compiled = None  # (key, nc)


def _spills(sizes):
    """Window spill (left, right) for classes laid out in this order.
    For class span [s,e): the left spill is the largest window start
    inside it minus s; the right spill is e minus the smallest window
    end inside it."""
    starts = np.concatenate([[0], np.cumsum(sizes)[:-1]])
    ends = starts + sizes
    wl = ((ends - 1) // P) * P  # largest window start < e
    spl = np.where(wl > starts, wl - starts, 0).max()
    wr = (starts // P + 1) * P  # smallest window end > s
    spr = np.where(wr < ends, ends - wr, 0).max()
    return int(spl), int(spr)


def _order_classes(cnt):
    """Anneal a class permutation so every 128-row window's class band
    fits in [win_start - 64, win_end + spr] with spr minimal (the 64-col
    left margin is forced by LDWEIGHTS alignment, so it is free)."""
    import random

    ncls = len(cnt)

    def score(perm):
        spl, spr = _spills(cnt[perm])
        return spr + (0 if spl <= 64 else 50 * (spl - 64))

    rng = random.Random(0)
    desc = sorted(range(ncls), key=lambda c: -cnt[c])
    perm = np.empty(ncls, np.int64)
    perm[0::2] = desc[: (ncls + 1) // 2]
    perm[1::2] = desc[(ncls + 1) // 2 :][::-1]
    best = perm.copy()
    bv = cv = score(perm)
    import math

    for it in range(40000):
        i = rng.randrange(ncls)
        j = rng.randrange(ncls)
        if i == j:
            continue
        perm[i], perm[j] = perm[j], perm[i]
        v = score(perm)
        T = 15.0 * (1 - it / 40000) + 0.2
        if v <= cv or rng.random() < math.exp((cv - v) / T):
            cv = v
            if v < bv:
                bv = v
                best = perm.copy()
        else:
            perm[i], perm[j] = perm[j], perm[i]
    return best


def _build_nc(SPL, W, NCOL):
    import concourse.mybir as mybir
    import concourse.tile as tile
    from concourse import bacc

    nc = bacc.Bacc("TRN2", target_bir_lowering=False)
    f32 = mybir.dt.float32
    bf16 = mybir.dt.bfloat16
    fp8 = mybir.dt.float8e4
    DR = mybir.MatmulPerfMode.DoubleRow
    X = mybir.AxisListType.X
    MIN = mybir.AluOpType.min

    # packed bf16 tensor: [lhsa (MB) | aug (NCOL) | eye (P) | psbp (MT*W)]
    LHSA = 0
    AUG = MB
    EYE = MB + NCOL
    PSBP = MB + NCOL + P
    PK = MB + NCOL + P + MT * W

    xt_d = nc.dram_tensor("xt", [P, KX, NCOL], fp8, kind="ExternalInput")
    pk_d = nc.dram_tensor("pk", [P, PK], bf16, kind="ExternalInput")
    res_d = nc.dram_tensor("res", [P, 2 * MT], f32, kind="ExternalOutput")

    with tile.TileContext(nc) as tc:
        with (
            tc.tile_pool(name="singles", bufs=1) as singles,
            tc.tile_pool(name="psa", bufs=1, space="PSUM") as psa,
        ):
            xt = singles.tile([P, KX, NCOL], fp8)
            pk = singles.tile([P, PK], bf16)
            # DMA bandwidth is shared across queues (~300 B/ns aggregate),
            # so prioritize xt in chunk order (PE consumes chunk-major);
            # the pk blocks are only needed at the end, psbp last of all
            nc.sync.dma_start(out=xt[:, 0:2, :], in_=xt_d[:, 0:2, :])
            nc.scalar.dma_start(out=xt[:, 2:4, :], in_=xt_d[:, 2:4, :])
            nc.gpsimd.dma_start(out=xt[:, 4:6, :], in_=xt_d[:, 4:6, :])
            nc.sync.dma_start(out=xt[:, 6:8, :], in_=xt_d[:, 6:8, :])
            nc.scalar.dma_start(out=xt[:, 8:10, :], in_=xt_d[:, 8:10, :])
            nc.gpsimd.dma_start(out=xt[:, 10:12, :], in_=xt_d[:, 10:12, :])
            nc.sync.dma_start(out=xt[:, 12:14, :], in_=xt_d[:, 12:14, :])
            nc.scalar.dma_start(out=xt[:, 14:KX, :], in_=xt_d[:, 14:KX, :])
            nc.scalar.dma_start(out=pk[:, 0:PSBP], in_=pk_d[:, 0:PSBP])
            nc.sync.dma_start(out=pk[:, PSBP:PK], in_=pk_d[:, PSBP:PK])

            fg = singles.tile([P, 2 * MT], f32, name="fg")

            at = [psa.tile([P, 512], f32, name=f"a{mt}") for mt in range(MT)]

            # chunk-major fp8 DR chains: the PE consumes chunk pairs in
            # DMA arrival order across all four PSUM banks (no mid stalls)
            def dr(c, mt):
                nc.tensor.matmul(
                    at[mt][:, 0:W],
                    xt[:, c : c + 2, SPL + mt * P : SPL + mt * P + P],
                    xt[:, c : c + 2, mt * P : mt * P + W],
                    start=(c == 0),
                    stop=False,
                    perf_mode=DR,
                )

            for c in range(0, KX, 2):
                for mt in range(MT):
                    dr(c, mt)

            def aug(mt):
                nc.tensor.matmul(
                    at[mt][:, 0:W],
                    pk[:, LHSA + mt * P : LHSA + (mt + 1) * P],
                    pk[:, AUG + mt * P : AUG + mt * P + W],
                    start=False,
                    stop=True,
                )

            def redmin(mt):
                # far stat: rowmin of psA (diag never wins the min)
                nc.vector.tensor_reduce(
                    fg[:, mt : mt + 1], at[mt][:, 0:W], axis=X, op=MIN
                )

            def eyeadd(mt):
                # after the f-reduce has read psA, accumulate the
                # 2C*mask + DIAG*diag term into the same PSUM bank on the
                # PE (WAR dep is tracked by the tile framework)
                nc.tensor.matmul(
                    at[mt][:, 0:W],
                    pk[:, EYE : EYE + P],
                    pk[:, PSBP + mt * W : PSBP + (mt + 1) * W],
                    start=False,
                    stop=True,
                )

            def redmax(mt):
                # near stat: plain rowmax of psA after the mask/diag add
                nc.vector.reduce_max(
                    fg[:, MT + mt : MT + mt + 1], at[mt][:, 0:W], axis=X
                )

            aug(0)
            aug(1)
            redmin(0)
            eyeadd(0)
            aug(2)
            redmin(1)
            eyeadd(1)
            aug(3)
            redmin(2)
            redmax(0)
            eyeadd(2)
            redmin(3)
            redmax(1)
            eyeadd(3)
            redmax(2)
            redmax(3)

            nc.sync.dma_start(out=res_d[:, :], in_=fg)

    nc.compile()
    return nc


def _prep(x, t):
    x = np.asarray(x, np.float32)
    t = np.asarray(t).astype(np.int64)
    cnt = np.bincount(t, minlength=NCLS)
    perm = _order_classes(cnt)
    rank = np.empty(NCLS, np.int64)
    rank[perm] = np.arange(NCLS)
    rt = rank[t]  # relabeled classes, contiguous in the annealed order
    order = np.argsort(rt, kind="stable")
    ts = rt[order]
    spl, spr = _spills(cnt[perm])
    SPL = ((spl + 63) // 64) * 64  # LDWEIGHTS needs 64-col-aligned lhs
    W = ((P + SPL + spr) + 7) // 8 * 8
    NCOL = (MB + (W - P) + 31) // 32 * 32  # chunk-pair offsets stay 64B-aligned

    q8 = (np.float32(np.sqrt(2.0)) * x).astype(F8)  # [N, D]
    sq = np.sum(x.astype(np.float64) ** 2, axis=1)
    sqhi = sq.astype(BF)
    sqlo = (sq - sqhi.astype(np.float64)).astype(BF)

    LHSA = 0
    AUG = MB
    EYE = MB + NCOL
    PSBP = MB + NCOL + P
    PK = MB + NCOL + P + MT * W

    in_maps = []
    meta = []
    for c0 in range(NCORES):
        u0 = c0 * MB - SPL
        uidx = np.arange(u0, u0 + NCOL)
        valid = (uidx >= 0) & (uidx < N)
        gu = order[np.clip(uidx, 0, N - 1)]
        tu = np.where(valid, ts[np.clip(uidx, 0, N - 1)], -1)

        xt_cols = q8[gu].T.copy()  # [D, NCOL]
        xt_cols[:, ~valid] = F8(0.0)
        xt_np = np.ascontiguousarray(
            xt_cols.reshape(KX, P, NCOL).transpose(1, 0, 2)
        )

        pk_np = np.zeros((P, PK), BF)
        # lhsa block: row0 = row1 = 1, rows 2+c = onehot(t_row)
        rows = order[c0 * MB : (c0 + 1) * MB]
        ohr = np.zeros((NCLS, MB), np.float32)
        ohr[rt[rows], np.arange(MB)] = 1.0
        pk_np[0, LHSA : LHSA + MB] = BF(1.0)
        pk_np[1, LHSA : LHSA + MB] = BF(1.0)
        pk_np[2 : 2 + NCLS, LHSA : LHSA + MB] = ohr.astype(BF)
        # aug block: row0 = -sqhi_j, row1 = -sqlo_j, rows 2+c = -C*onehot
        pk_np[0, AUG : AUG + NCOL] = np.where(valid, -sqhi[gu], BF(0.0))
        pk_np[1, AUG : AUG + NCOL] = np.where(valid, -sqlo[gu], BF(0.0))
        oh = np.zeros((NCLS, NCOL), np.float32)
        oh[tu[valid], np.nonzero(valid)[0]] = 1.0
        pk_np[2 : 2 + NCLS, AUG : AUG + NCOL] = (-C * oh).astype(BF)
        # psbp blocks: per tile, 2C*mask with diag overwritten to DIAG
        for mt in range(MT):
            tr = tu[SPL + mt * P : SPL + mt * P + P]  # row classes
            tc_ = tu[mt * P : mt * P + W]  # window col classes
            m = (tr[:, None] == tc_[None, :]) & (tr[:, None] >= 0)
            blk = np.where(m, np.float32(2.0 * C), np.float32(0.0))
            blk[np.arange(P), SPL + np.arange(P)] = np.float32(DIAG)
            pk_np[:, PSBP + mt * W : PSBP + (mt + 1) * W] = blk.astype(BF)
        pk_np[np.arange(P), EYE + np.arange(P)] = BF(1.0)

        in_maps.append({"xt": xt_np, "pk": pk_np})
        meta.append(rows)
    return in_maps, meta, sq, (SPL, W, NCOL)


def _assemble(results, meta, sq):
    far2 = np.empty(N, np.float64)
    near2 = np.empty(N, np.float64)
    for c0 in range(NCORES):
        r = np.asarray(results[c0]["res"], np.float64)  # [P, 2*MT]
        rows = meta[c0]
        for mt in range(MT):
            g = rows[mt * P : (mt + 1) * P]
            far2[g] = sq[g] - C - r[:, mt]
            near2[g] = sq[g] + C - r[:, MT + mt]
    far = np.sqrt(np.maximum(far2, 0.0))
    near = np.sqrt(np.maximum(near2, 0.0))
    loss = np.float32(np.mean(np.maximum(far - near, 0.0)))
    return np.asarray(loss, np.float32)


def run_kernel(inputs, targets, trace=False):
    """Returns (loss, BassKernelResults)."""
    from concourse.bass_utils import run_bass_kernel_spmd

    global _compiled
    in_maps, meta, sq, key = _prep(inputs, targets)
    if _compiled is None or _compiled[0] != key:
        _compiled = (key, _build_nc(*key))
    nc = _compiled[1]
    br = run_bass_kernel_spmd(
        nc, in_maps, core_ids=list(range(NCORES)), trace=trace
    )
    return _assemble(br.results, meta, sq), br


def kernel(inputs, targets):
    loss, _ = run_kernel(inputs, targets)
    return loss


# revision 25
# speedup vs baseline: 1.0144x; 1.0144x over previous
"""Trainium2 Bass kernel for a pairwise-distance cluster margin loss.

Math (matches the jax reference):
    far_i  = max_{j: t_j=t_i} dist_ij
    near_i = second smallest dist_ij over class(i)  (smallest is self)
    loss   = mean(relu(far - near))

Key insight: the loss only involves SAME-CLASS distances.  With rows
sorted by class, each 128-row tile's class-mates lie within a narrow
band of the sorted order (max class size ~82), so each tile only needs
W ~ 208 columns instead of 4096 -> ~20x less GEMM work than the full
distance matrix.  The class order is annealed on the host so every
128-row window's class band fits in [win-64, win+W-128).

Per core (512 sorted rows): the column "universe" is the sorted slice
order[512c-SPL : 512c-SPL+NCOL] (padded with zeros at the array ends).
Row-tile mt multiplies against universe cols [128mt, 128mt+W).  A single
fp8 tensor xt8 = fp8(sqrt2*x[universe])^T serves as BOTH matmul operands
(lhsT slice = own rows, rhs slice = window), so the PE computes
    psA = 2 x_i.x_j - sq_j - C*mask      (fp8 DR chain + one bf16 aug)
and the stats flip max<->min versus the usual formulation:
    rowmin(psA)                   -> far2  = sq_i - C - fstat
    rowmax(psA + 2C*mask + Ddiag) -> near2 = sq_i + C - gstat
The chains are issued chunk-major so the PE consumes chunk pairs in DMA
arrival order (inputs stream just-in-time on three engine queues); after
each tile's f-reduce, one eye-matmul accumulates the host-precomputed
2C*mask + DIAG*diag block into the same PSUM bank (WAR dependency), so
the near stat is a plain rowmax and the DVE does only two reduces per
tile.  Host applies sqrt / relu / mean on the 4096 reduced stats.
"""

import numpy as np
import ml_dtypes

BF = ml_dtypes.bfloat16
F8 = ml_dtypes.float8_e4m3

N = 4096  # rows (points)
D = 2048  # feature dim
P = 128  # partitions
NCORES = 8
MB = N // NCORES  # 512 rows per core
KX = D // P  # 16 x-chunks of 128
MT = MB // P  # 4 row tiles of 128 per core
NCLS = 64

C = float(2.0**17)  # mask offset; > max |2xixj - sqj| (~15k)
DIAG = -float(2.0**31)  # diagonal push-out

_compiled = None  # (key, nc)


def _spills(sizes):
    """Window spill (left, right) for classes laid out in this order.
    For class span [s,e): the left spill is the largest window start
    inside it minus s; the right spill is e minus the smallest window
    end inside it."""
    starts = np.concatenate([[0], np.cumsum(sizes)[:-1]])
    ends = starts + sizes
    wl = ((ends - 1) // P) * P  # largest window start < e
    spl = np.where(wl > starts, wl - starts, 0).max()
    wr = (starts // P + 1) * P  # smallest window end > s
    spr = np.where(wr < ends, ends - wr, 0).max()
    return int(spl), int(spr)


def _order_classes(cnt):
    """Anneal a class permutation so every 128-row window's class band
    fits in [win_start - 64, win_end + spr] with spr minimal (the 64-col
    left margin is forced by LDWEIGHTS alignment, so it is free)."""
    import random

    ncls = len(cnt)

    def score(perm):
        spl, spr = _spills(cnt[perm])
        return spr + (0 if spl <= 64 else 50 * (spl - 64))

    rng = random.Random(0)
    desc = sorted(range(ncls), key=lambda c: -cnt[c])
    perm = np.empty(ncls, np.int64)
    perm[0::2] = desc[: (ncls + 1) // 2]
    perm[1::2] = desc[(ncls + 1) // 2 :][::-1]
    best = perm.copy()
    bv = cv = score(perm)
    import math

    for it in range(40000):
        i = rng.randrange(ncls)
        j = rng.randrange(ncls)
        if i == j:
            continue
        perm[i], perm[j] = perm[j], perm[i]
        v = score(perm)
        T = 15.0 * (1 - it / 40000) + 0.2
        if v <= cv or rng.random() < math.exp((cv - v) / T):
            cv = v
            if v < bv:
                bv = v
                best = perm.copy()
        else:
            perm[i], perm[j] = perm[j], perm[i]
    return best


def _build_nc(SPL, W, NCOL):
    import concourse.mybir as mybir
    import concourse.tile as tile
    from concourse import bacc

    nc = bacc.Bacc("TRN2", target_bir_lowering=False)
    f32 = mybir.dt.float32
    bf16 = mybir.dt.bfloat16
    fp8 = mybir.dt.float8e4
    DR = mybir.MatmulPerfMode.DoubleRow
    X = mybir.AxisListType.X
    MIN = mybir.AluOpType.min

    # packed bf16 tensor: [lhsa (MB) | aug (NCOL) | eye (P) | psbp (MT*W)]
    LHSA = 0
    AUG = MB
    EYE = MB + NCOL
    PSBP = MB + NCOL + P
    PK = MB + NCOL + P + MT * W

    xt_d = nc.dram_tensor("xt", [P, KX, NCOL], fp8, kind="ExternalInput")
    pk_d = nc.dram_tensor("pk", [P, PK], bf16, kind="ExternalInput")
    res_d = nc.dram_tensor("res", [P, 2 * MT], f32, kind="ExternalOutput")

    with tile.TileContext(nc) as tc:
        with (
            tc.tile_pool(name="singles", bufs=1) as singles,
            tc.tile_pool(name="psa", bufs=1, space="PSUM") as psa,
        ):
            xt = singles.tile([P, KX, NCOL], fp8)
            pk = singles.tile([P, PK], bf16)
            # DMA bandwidth is shared across queues (~300 B/ns aggregate),
            # so prioritize xt in chunk order (PE consumes chunk-major);
            # the pk blocks are only needed at the end, psbp last of all
            nc.sync.dma_start(out=xt[:, 0:2, :], in_=xt_d[:, 0:2, :])
            nc.scalar.dma_start(out=xt[:, 2:4, :], in_=xt_d[:, 2:4, :])
            nc.gpsimd.dma_start(out=xt[:, 4:6, :], in_=xt_d[:, 4:6, :])
            nc.sync.dma_start(out=xt[:, 6:8, :], in_=xt_d[:, 6:8, :])
            nc.scalar.dma_start(out=xt[:, 8:10, :], in_=xt_d[:, 8:10, :])
            nc.gpsimd.dma_start(out=xt[:, 10:12, :], in_=xt_d[:, 10:12, :])
            nc.sync.dma_start(out=xt[:, 12:14, :], in_=xt_d[:, 12:14, :])
            nc.scalar.dma_start(out=xt[:, 14:KX, :], in_=xt_d[:, 14:KX, :])
            nc.scalar.dma_start(out=pk[:, 0:PSBP], in_=pk_d[:, 0:PSBP])
            nc.sync.dma_start(out=pk[:, PSBP:PK], in_=pk_d[:, PSBP:PK])

            fg = singles.tile([P, 2 * MT], f32, name="fg")

            at = [psa.tile([P, 512], f32, name=f"a{mt}") for mt in range(MT)]

            # chunk-major fp8 DR chains: the PE consumes chunk pairs in
            # DMA arrival order across all four PSUM banks (no mid stalls)
            def dr(c, mt):
                nc.tensor.matmul(
                    at[mt][:, 0:W],
                    xt[:, c : c + 2, SPL + mt * P : SPL + mt * P + P],
                    xt[:, c : c + 2, mt * P : mt * P + W],
                    start=(c == 0),
                    stop=False,
                    perf_mode=DR,
                )

            for c in range(0, KX, 2):
                for mt in range(MT):
                    dr(c, mt)

            def aug(mt):
                nc.tensor.matmul(
                    at[mt][:, 0:W],
                    pk[:, LHSA + mt * P : LHSA + (mt + 1) * P],
                    pk[:, AUG + mt * P : AUG + mt * P + W],
                    start=False,
                    stop=True,
                )

            def redmin(mt):
                # far stat: rowmin of psA (diag never wins the min)
                nc.vector.tensor_reduce(
                    fg[:, mt : mt + 1], at[mt][:, 0:W], axis=X, op=MIN
                )

            def eyeadd(mt):
                # after the f-reduce has read psA, accumulate the
                # 2C*mask + DIAG*diag term into the same PSUM bank on the
                # PE (WAR dep is tracked by the tile framework)
                nc.tensor.matmul(
                    at[mt][:, 0:W],
                    pk[:, EYE : EYE + P],
                    pk[:, PSBP + mt * W : PSBP + (mt + 1) * W],
                    start=False,
                    stop=True,
                )

            def redmax(mt):
                # near stat: plain rowmax of psA after the mask/diag add
                nc.vector.reduce_max(
                    fg[:, MT + mt : MT + mt + 1], at[mt][:, 0:W], axis=X
                )

            aug(0)
            aug(1)
            redmin(0)
            eyeadd(0)
            aug(2)
            redmin(1)
            eyeadd(1)
            aug(3)
            redmin(2)
            redmax(0)
            eyeadd(2)
            redmin(3)
            redmax(1)
            eyeadd(3)
            redmax(2)
            redmax(3)

            nc.sync.dma_start(out=res_d[:, :], in_=fg)

    nc.compile()
    return nc


def _prep(x, t):
    x = np.asarray(x, np.float32)
    t = np.asarray(t).astype(np.int64)
    cnt = np.bincount(t, minlength=NCLS)
    perm = _order_classes(cnt)
    rank = np.empty(NCLS, np.int64)
    rank[perm] = np.arange(NCLS)
    rt = rank[t]  # relabeled classes, contiguous in the annealed order
    order = np.argsort(rt, kind="stable")
    ts = rt[order]
    spl, spr = _spills(cnt[perm])
    SPL = ((spl + 63) // 64) * 64  # LDWEIGHTS needs 64-col-aligned lhs
    W = ((P + SPL + spr) + 7) // 8 * 8
    NCOL = (MB + (W - P) + 31) // 32 * 32  # chunk-pair offsets stay 64B-aligned

    q8 = (np.float32(np.sqrt(2.0)) * x).astype(F8)  # [N, D]
    sq = np.sum(x.astype(np.float64) ** 2, axis=1)
    sqhi = sq.astype(BF)
    sqlo = (sq - sqhi.astype(np.float64)).astype(BF)

    LHSA = 0
    AUG = MB
    EYE = MB + NCOL
    PSBP = MB + NCOL + P
    PK = MB + NCOL + P + MT * W

    in_maps = []
    meta = []
    for c0 in range(NCORES):
        u0 = c0 * MB - SPL
        uidx = np.arange(u0, u0 + NCOL)
        valid = (uidx >= 0) & (uidx < N)
        gu = order[np.clip(uidx, 0, N - 1)]
        tu = np.where(valid, ts[np.clip(uidx, 0, N - 1)], -1)

        xt_cols = q8[gu].T.copy()  # [D, NCOL]
        xt_cols[:, ~valid] = F8(0.0)
        xt_np = np.ascontiguousarray(
            xt_cols.reshape(KX, P, NCOL).transpose(1, 0, 2)
        )

        pk_np = np.zeros((P, PK), BF)
        # lhsa block: row0 = row1 = 1, rows 2+c = onehot(t_row)
        rows = order[c0 * MB : (c0 + 1) * MB]
        ohr = np.zeros((NCLS, MB), np.float32)
        ohr[rt[rows], np.arange(MB)] = 1.0
        pk_np[0, LHSA : LHSA + MB] = BF(1.0)
        pk_np[1, LHSA : LHSA + MB] = BF(1.0)
        pk_np[2 : 2 + NCLS, LHSA : LHSA + MB] = ohr.astype(BF)
        # aug block: row0 = -sqhi_j, row1 = -sqlo_j, rows 2+c = -C*onehot
        pk_np[0, AUG : AUG + NCOL] = np.where(valid, -sqhi[gu], BF(0.0))
        pk_np[1, AUG : AUG + NCOL] = np.where(valid, -sqlo[gu], BF(0.0))
        oh = np.zeros((NCLS, NCOL), np.float32)
        oh[tu[valid], np.nonzero(valid)[0]] = 1.0
        pk_np[2 : 2 + NCLS, AUG : AUG + NCOL] = (-C * oh).astype(BF)
        # psbp blocks: per tile, 2C*mask with diag overwritten to DIAG
        for mt in range(MT):
            tr = tu[SPL + mt * P : SPL + mt * P + P]  # row classes
            tc_ = tu[mt * P : mt * P + W]  # window col classes
            m = (tr[:, None] == tc_[None, :]) & (tr[:, None] >= 0)
            blk = np.where(m, np.float32(2.0 * C), np.float32(0.0))
            blk[np.arange(P), SPL + np.arange(P)] = np.float32(DIAG)
            pk_np[:, PSBP + mt * W : PSBP + (mt + 1) * W] = blk.astype(BF)
        pk_np[np.arange(P), EYE + np.arange(P)] = BF(1.0)

        in_maps.append({"xt": xt_np, "pk": pk_np})
        meta.append(rows)
    return in_maps, meta, sq, (SPL, W, NCOL)


def _assemble(results, meta, sq):
    far2 = np.empty(N, np.float64)
    near2 = np.empty(N, np.float64)
    for c0 in range(NCORES):
        r = np.asarray(results[c0]["res"], np.float64)  # [P, 2*MT]
        rows = meta[c0]
        for mt in range(MT):
            g = rows[mt * P : (mt + 1) * P]
            far2[g] = sq[g] - C - r[:, mt]
            near2[g] = sq[g] + C - r[:, MT + mt]
    far = np.sqrt(np.maximum(far2, 0.0))
    near = np.sqrt(np.maximum(near2, 0.0))
    loss = np.float32(np.mean(np.maximum(far - near, 0.0)))
    return np.asarray(loss, np.float32)


def run_kernel(inputs, targets, trace=False):
    """Returns (loss, BassKernelResults)."""
    from concourse.bass_utils import run_bass_kernel_spmd

    global _compiled
    in_maps, meta, sq, key = _prep(inputs, targets)
    if _compiled is None or _compiled[0] != key:
        _compiled = (key, _build_nc(*key))
    nc = _compiled[1]
    br = run_bass_kernel_spmd(
        nc, in_maps, core_ids=list(range(NCORES)), trace=trace
    )
    return _assemble(br.results, meta, sq), br


def kernel(inputs, targets):
    loss, _ = run_kernel(inputs, targets)
    return loss
